# revision 1
# baseline (speedup 1.0000x reference)
"""nn_CNUs kernel for 8 TRN2 NeuronCores.

Pipeline (all FLOPs of the reference run on-device):
  Kernel A (q-sharded): L2-normalize K rows and x rows; split normalized
    values into bf16 hi/lo pairs (exact to ~2^-17).
  Host glue: pure layout work (transposes, concatenation, dtype casts of M,
    ones-column append, sharding).
  Kernel B (batch-sharded, 128 rows/core): for each q:
    - responses via 2 stacked-bf16 matmuls (all 4 hi/lo cross terms,
      fp32 PSUM accumulation -> fp32-accurate responses)
    - top-16 threshold T via segmented max8 + candidate top-16 (DVE)
    - exact 0/1 selection mask via ACT sigmoid(2^30*(r-T)+eps) -> fp8
    - xbar DMA-transpose of the mask (fp16-pair view)
    - combine: mask @ [M|1] in fp8 x fp16 matmul, normalize by the count
      column (softmax at temperature 0.0125/ sqrt-free uniform weighting;
      the temperature-induced deviation is ~1e-4 of the output scale).
  Host fixup: rows whose selection count != 16 (candidate-miss or ties,
    ~1e-4 of rows) are recomputed with the exact reference formula.
"""
import sys
if '/opt/trn_rl_repo' not in sys.path:
    sys.path.insert(0, '/opt/trn_rl_repo')

import numpy as np
import ml_dtypes

import concourse.bacc as bacc
import concourse.mybir as mybir
import concourse.tile as tile
from concourse.bass_utils import run_bass_kernel_spmd

N_CORES = 8
BF, D, Q, MK, DELTA = 1024, 64, 32, 4096, 16
B = BF // N_CORES          # 128 batch rows per core (kernel B)
QS = Q // N_CORES          # 4 q per core (kernel A)
RPC = QS * MK              # K rows per core in kernel A
G = RPC // 128
NCH, CH, U1 = 8, 512, 65
SCALE = float(2 ** 30)
S_TEMP = 0.1 / 8.0         # gamma_alpha / sqrt(D)

_cache = {}


# ----------------------------------------------------------------- kernel A
def _build_a():
    nc = bacc.Bacc("TRN2", target_bir_lowering=False, debug=False,
                   num_devices=N_CORES)
    k_d = nc.dram_tensor("Kc", [RPC, D], mybir.dt.float32, kind="ExternalInput")
    x_d = nc.dram_tensor("xc", [B, D], mybir.dt.float32, kind="ExternalInput")
    kh_d = nc.dram_tensor("Knh", [RPC, D], mybir.dt.bfloat16, kind="ExternalOutput")
    kl_d = nc.dram_tensor("Knl", [RPC, D], mybir.dt.bfloat16, kind="ExternalOutput")
    xh_d = nc.dram_tensor("xnh", [B, D], mybir.dt.bfloat16, kind="ExternalOutput")
    xl_d = nc.dram_tensor("xnl", [B, D], mybir.dt.bfloat16, kind="ExternalOutput")

    with tile.TileContext(nc) as tc:
        with tc.tile_pool(name="sbuf", bufs=1) as pool:
            k32 = pool.tile([128, G * D], mybir.dt.float32)
            nc.sync.dma_start(out=k32[:, :], in_=k_d.ap())
            ksq = pool.tile([128, G * D], mybir.dt.float32)
            nc.scalar.activation(ksq[:, :], k32[:, :],
                                 mybir.ActivationFunctionType.Square)
            ssq = pool.tile([128, G], mybir.dt.float32)
            nc.vector.tensor_reduce(
                ssq[:, :], ksq[:, :].rearrange("p (g d) -> p g d", g=G),
                axis=mybir.AxisListType.X, op=mybir.AluOpType.add,
                apply_absolute_value=False, negate=False)
            srt = pool.tile([128, G], mybir.dt.float32)
            nc.scalar.activation(srt[:, :], ssq[:, :],
                                 mybir.ActivationFunctionType.Sqrt)
            rn = pool.tile([128, G], mybir.dt.float32)
            nc.vector.reciprocal(rn[:, :], srt[:, :])
            kn32 = pool.tile([128, G * D], mybir.dt.float32)
            for g in range(G):
                nc.vector.tensor_scalar_mul(
                    kn32[:, g * D:(g + 1) * D], k32[:, g * D:(g + 1) * D],
                    rn[:, g:g + 1])
            knh = pool.tile([128, G * D], mybir.dt.bfloat16)
            nc.scalar.activation(knh[:, :], kn32[:, :],
                                 mybir.ActivationFunctionType.Copy)
            knl = pool.tile([128, G * D], mybir.dt.bfloat16)
            nc.gpsimd.tensor_sub(knl[:, :], kn32[:, :], knh[:, :])
            nc.sync.dma_start(out=kh_d.ap(), in_=knh[:, :])
            nc.sync.dma_start(out=kl_d.ap(), in_=knl[:, :])

            x32 = pool.tile([128, D], mybir.dt.float32)
            nc.sync.dma_start(out=x32[:, :], in_=x_d.ap())
            xsq = pool.tile([128, D], mybir.dt.float32)
            nc.scalar.activation(xsq[:, :], x32[:, :],
                                 mybir.ActivationFunctionType.Square)
            xssq = pool.tile([128, 1], mybir.dt.float32)
            nc.vector.tensor_reduce(
                xssq[:, :], xsq[:, :].rearrange("p (g d) -> p g d", g=1),
                axis=mybir.AxisListType.X, op=mybir.AluOpType.add,
                apply_absolute_value=False, negate=False)
            xsrt = pool.tile([128, 1], mybir.dt.float32)
            nc.scalar.activation(xsrt[:, :], xssq[:, :],
                                 mybir.ActivationFunctionType.Sqrt)
            xrn = pool.tile([128, 1], mybir.dt.float32)
            nc.vector.reciprocal(xrn[:, :], xsrt[:, :])
            xn32 = pool.tile([128, D], mybir.dt.float32)
            nc.vector.tensor_scalar_mul(xn32[:, :], x32[:, :], xrn[:, :])
            xnh = pool.tile([128, D], mybir.dt.bfloat16)
            nc.scalar.activation(xnh[:, :], xn32[:, :],
                                 mybir.ActivationFunctionType.Copy)
            xnl = pool.tile([128, D], mybir.dt.bfloat16)
            nc.gpsimd.tensor_sub(xnl[:, :], xn32[:, :], xnh[:, :])
            nc.sync.dma_start(out=xh_d.ap(), in_=xnh[:, :])
            nc.sync.dma_start(out=xl_d.ap(), in_=xnl[:, :])
    nc.compile()
    return nc


# ----------------------------------------------------------------- kernel B
def _build_b():
    nc = bacc.Bacc("TRN2", target_bir_lowering=False, debug=False,
                   num_devices=N_CORES)
    xa_d = nc.dram_tensor("xa", [128, B], mybir.dt.bfloat16, kind="ExternalInput")
    xb_d = nc.dram_tensor("xb", [128, B], mybir.dt.bfloat16, kind="ExternalInput")
    knt_d = nc.dram_tensor("KnT", [Q, 128, MK], mybir.dt.bfloat16, kind="ExternalInput")
    mp_d = nc.dram_tensor("Mp", [Q, 128, 32 * U1], mybir.dt.float16, kind="ExternalInput")
    w_d = nc.dram_tensor("W", [B, Q * 64], mybir.dt.float32, kind="ExternalOutput")
    cnt_d = nc.dram_tensor("cnt", [B, Q], mybir.dt.float32, kind="ExternalOutput")

    with tile.TileContext(nc) as tc:
        with tc.tile_pool(name="const", bufs=1) as cpool, \
             tc.tile_pool(name="io", bufs=1) as iopool, \
             tc.tile_pool(name="knt", bufs=3) as kpool, \
             tc.tile_pool(name="mp", bufs=3) as mpool, \
             tc.tile_pool(name="mask", bufs=3) as maskpool, \
             tc.tile_pool(name="sel", bufs=2) as selpool, \
             tc.tile_pool(name="ps", bufs=8, space="PSUM") as psum:

            xa = cpool.tile([128, B], mybir.dt.bfloat16)
            xb = cpool.tile([128, B], mybir.dt.bfloat16)
            nc.sync.dma_start(out=xa[:, :], in_=xa_d.ap())
            nc.sync.dma_start(out=xb[:, :], in_=xb_d.ap())
            wsb = iopool.tile([B, Q * 64], mybir.dt.float32, tag="wout")
            csb = iopool.tile([B, Q], mybir.dt.float32, tag="cout")

            def emit_mm2(prev_mT, prev_mp, wp):
                mT8 = prev_mT[:, :].bitcast(mybir.dt.float8e4)
                k = 0
                for t in range(16):
                    for j in range(2):
                        lhsT = mT8[:, 256 * t:256 * (t + 1)].rearrange(
                            "p (b two) -> p b two", two=2)[:, :, j:j + 1]
                        rhs = prev_mp[:, (t * 2 + j) * U1:(t * 2 + j + 1) * U1]
                        nc.tensor.matmul(wp[:, :U1], lhsT, rhs,
                                         start=(k == 0), stop=(k == 31))
                        k += 1

            def emit_epilogue(wp, prev_q):
                # count != 16 rows are host-fixed, so scale by 1/16 always
                nc.scalar.activation(wsb[:, 64 * prev_q:64 * (prev_q + 1)],
                                     wp[:, 0:64],
                                     mybir.ActivationFunctionType.Copy,
                                     scale=1.0 / 16.0)
                nc.scalar.activation(csb[:, prev_q:prev_q + 1], wp[:, 64:65],
                                     mybir.ActivationFunctionType.Copy)

            pend = []
            for q in range(Q):
                knt = kpool.tile([128, MK], mybir.dt.bfloat16, tag="knt")
                nc.sync.dma_start(out=knt[:, :], in_=knt_d.ap()[q])
                mp = mpool.tile([128, 32 * U1], mybir.dt.float16, tag="mp")
                nc.sync.dma_start(out=mp[:, :], in_=mp_d.ap()[q])

                chunks = []
                cands = selpool.tile([B, 64], mybir.dt.float32, tag="cands")
                for c in range(NCH):
                    rp = psum.tile([B, CH], mybir.dt.float32, tag="bank")
                    nc.tensor.matmul(rp[:, :], xa[:, :],
                                     knt[:, CH * c:CH * (c + 1)],
                                     start=True, stop=False)
                    nc.tensor.matmul(rp[:, :], xb[:, :],
                                     knt[:, CH * c:CH * (c + 1)],
                                     start=False, stop=True)
                    nc.vector.max(cands[:, 8 * c:8 * (c + 1)], rp[:, :])
                    chunks.append(rp)

                v1 = selpool.tile([B, 8], mybir.dt.float32, tag="v1")
                nc.vector.max(v1[:, :], cands[:, :])
                candr = selpool.tile([B, 64], mybir.dt.float32, tag="candr")
                nc.vector.match_replace(candr[:, :], v1[:, :], cands[:, :], -1e30)
                v2 = selpool.tile([B, 8], mybir.dt.float32, tag="v2")
                nc.vector.max(v2[:, :], candr[:, :])
                bt = selpool.tile([B, 1], mybir.dt.float32, tag="bt")
                nc.vector.tensor_scalar(bt[:, :], v2[:, 7:8], -SCALE, 37.0,
                                        op0=mybir.AluOpType.mult,
                                        op1=mybir.AluOpType.add)

                # mask chunk 0 first so the combine from two iterations ago
                # (which reuses chunk 0's PSUM bank in place) can start early.
                # mm2 consumes maskT from q-2, which is guaranteed complete,
                # so the PE never stalls on the transpose DMA.
                # c7's mask first (DVE is_ge) so the q-2 combine can take
                # over its bank immediately; the epilogue (ACT, after the
                # sigmoids) releases it just before mm1(q+1) needs it last.
                mask8 = maskpool.tile([B, MK], mybir.dt.float8e4, tag="mask8")
                for c in (7, 6):
                    nc.vector.tensor_scalar(mask8[:, CH * c:CH * (c + 1)],
                                            chunks[c][:, :], v2[:, 7:8], None,
                                            op0=mybir.AluOpType.is_ge)
                if len(pend) == 2:
                    pmT, pmp, pq = pend.pop(0)
                    emit_mm2(pmT, pmp, chunks[7])
                else:
                    pq = None
                for c in range(6):
                    nc.scalar.activation(mask8[:, CH * c:CH * (c + 1)],
                                         chunks[c][:, :],
                                         mybir.ActivationFunctionType.Sigmoid,
                                         bias=bt[:, 0:1], scale=SCALE)
                if pq is not None:
                    emit_epilogue(chunks[7], pq)

                m16 = mask8[:, :].bitcast(mybir.dt.float16)
                mT = maskpool.tile([128, 2048], mybir.dt.float16, tag="maskT")
                nc.scalar.dma_start_transpose(
                    mT[:, :].rearrange("p (t b) -> p t b", t=16), m16[:, :])
                pend.append((mT, mp, q))

            for pmT, pmp, pq in pend:
                wp_last = psum.tile([B, CH], mybir.dt.float32, tag="bank")
                emit_mm2(pmT, pmp, wp_last)
                emit_epilogue(wp_last, pq)

            nc.sync.dma_start(out=w_d.ap(), in_=wsb[:, :])
            nc.sync.dma_start(out=cnt_d.ap(), in_=csb[:, :])
    nc.compile()
    return nc


def _get(name, builder):
    if name not in _cache:
        _cache[name] = builder()
    return _cache[name]


# -------------------------------------------------------------- host fixup
def _fixup_rows(W, cnt, x, K, M):
    """Recompute rows whose on-device selection count != 16 with the exact
    reference formula (fp32)."""
    bad = np.argwhere(np.abs(cnt - 16.0) > 0.25)
    if len(bad) == 0:
        return W
    xf = np.asarray(x, np.float32)
    Kf = np.asarray(K, np.float32)
    Mf = np.asarray(M, np.float32)
    for b, q in bad:
        xb = xf[b]
        xb = xb / max(np.sqrt(np.sum(xb * xb)), 1e-12)
        Kq = Kf[q]
        nrm = np.maximum(np.sqrt(np.sum(Kq * Kq, axis=1)), 1e-12)
        r = (Kq @ xb) / nrm
        idx = np.argsort(-r, kind="stable")[:DELTA]
        tr = r[idx]
        a = np.exp(S_TEMP * (tr - tr.max()))
        a /= a.sum()
        W[b, q] = (a[:, None] * Mf[q][idx]).sum(0)
    return W


def _run_spmd(nc, in_maps, trace):
    try:
        return run_bass_kernel_spmd(nc, in_maps, core_ids=list(range(N_CORES)),
                                    trace=trace)
    except Exception:
        # transient NRT device errors recover on retry
        return run_bass_kernel_spmd(nc, in_maps, core_ids=list(range(N_CORES)),
                                    trace=trace)


# ------------------------------------------------------------------- main
def _run(x, K, M, trace=False):
    x = np.ascontiguousarray(np.asarray(x, np.float32))
    K = np.ascontiguousarray(np.asarray(K, np.float32))
    M = np.ascontiguousarray(np.asarray(M, np.float32))

    nca = _get("a", _build_a)
    in_a = []
    for c in range(N_CORES):
        in_a.append({"Kc": K[c * QS:(c + 1) * QS].reshape(RPC, D),
                     "xc": x[c * B:(c + 1) * B]})
    res_a = _run_spmd(nca, in_a, trace)
    Knh = np.concatenate([np.asarray(o["Knh"]).reshape(QS, MK, D)
                          for o in res_a.results])
    Knl = np.concatenate([np.asarray(o["Knl"]).reshape(QS, MK, D)
                          for o in res_a.results])
    xnh = np.concatenate([np.asarray(o["xnh"]) for o in res_a.results])
    xnl = np.concatenate([np.asarray(o["xnl"]) for o in res_a.results])

    # host layout glue (no math): transposes, stacking, M cast + ones column
    KnT = np.stack([np.concatenate([Knh[q].T, Knl[q].T], 0) for q in range(Q)])
    M16 = M.astype(np.float16)
    ones = np.ones((MK, 1), np.float16)
    Mp = np.stack([
        np.concatenate([M16[q], ones], 1)
        .reshape(16, 128, 2, U1).transpose(1, 0, 2, 3).reshape(128, 32 * U1)
        for q in range(Q)])

    ncb = _get("b", _build_b)
    in_b = []
    for c in range(N_CORES):
        sl = slice(c * B, (c + 1) * B)
        in_b.append({"xa": np.concatenate([xnh[sl].T, xnl[sl].T], 0),
                     "xb": np.concatenate([xnl[sl].T, xnh[sl].T], 0),
                     "KnT": KnT, "Mp": Mp})
    res_b = _run_spmd(ncb, in_b, trace)
    W = np.concatenate([np.asarray(r["W"], np.float32).reshape(B, Q, 64)
                        for r in res_b.results])
    cnt = np.concatenate([np.asarray(r["cnt"], np.float32)
                          for r in res_b.results])

    W = _fixup_rows(W, cnt, x, K, M)
    return W, res_a.exec_time_ns, res_b.exec_time_ns


def kernel(x, K, M):
    W, _, _ = _run(x, K, M, trace=False)
    return W



# revision 7
# speedup vs baseline: 1.1592x; 1.1592x over previous
"""nn_CNUs kernel for 8 TRN2 NeuronCores — single merged q-sharded kernel.

Sharding: each core owns 4 of 32 q-neurons and processes ALL 1024 batch rows
(vs. the old batch-sharded 2-kernel pipeline that replicated 51MB of K/M DMA
per core and serialized normalize->host->combine).

Per core, per q: on-device L2-normalize K rows, split into interleaved
bf16 hi/lo [d_hi|d_lo] layout, xbar-transpose to [128, 4096] (contraction
layout). Per unit (q, 128-batch tile): responses via 2 stacked-bf16 matmuls
per 512-chunk (fp32-exact), DVE max8 screen -> top-16 threshold, masks via
ACT sigmoid / gpsimd is_ge into fp8, xbar mask transpose (SP queue), combine
mask @ [M|1] two units later interleaved into a just-masked PSUM bank.
Host does layout only (reshapes, fp16 cast, permutation gathers) + fixup of
rows whose selection count != 16 (ties/candidate misses, ~1e-4).
"""
import sys
if '/opt/trn_rl_repo' not in sys.path:
    sys.path.insert(0, '/opt/trn_rl_repo')

import numpy as np
import ml_dtypes

import concourse.bacc as bacc
import concourse.mybir as mybir
import concourse.tile as tile
from concourse.bass_utils import run_bass_kernel_spmd

N_CORES = 8
BF, D, Q, MK, DELTA = 1024, 64, 32, 4096, 16
QS = Q // N_CORES          # 4 q per core
NBT = 8                    # batch tiles of 128 per core
NCH, CH, U1 = 8, 512, 65
SCALE = float(2 ** 30)
S_TEMP = 0.1 / 8.0         # gamma_alpha / sqrt(D)
AF = mybir.ActivationFunctionType
ALU = mybir.AluOpType

_cache = {}

# knt column c holds K-row m_col(c) = 32*(c%128) + c//128 (from the
# contiguous [128p x 32 rows] SBUF fill + 128-blocked xbar transpose).
_MCOL = (32 * (np.arange(MK) % 128) + np.arange(MK) // 128).astype(np.int64)
# mm2 chunk c2 (=2t+j), partition p contracts mask fp8 column 2*(128*t+p)+j.
_t2 = np.arange(32) // 2
_j2 = np.arange(32) % 2
_MP_IDX = _MCOL[2 * (128 * _t2[None, :] + np.arange(128)[:, None]) + _j2[None, :]]


def _build():
    nc = bacc.Bacc("TRN2", target_bir_lowering=False, debug=False,
                   num_devices=N_CORES)
    x_d = nc.dram_tensor("xr", [128, NBT * D], mybir.dt.float32, kind="ExternalInput")
    k_d = nc.dram_tensor("Kc", [QS, 128, 32 * D], mybir.dt.float32, kind="ExternalInput")
    mp_d = nc.dram_tensor("Mp", [QS, 128, 32 * U1], mybir.dt.float16, kind="ExternalInput")
    w_d = nc.dram_tensor("W", [128, QS * NBT * 64], mybir.dt.float32, kind="ExternalOutput")
    cnt_d = nc.dram_tensor("cnt", [128, QS * NBT], mybir.dt.float32, kind="ExternalOutput")

    with tile.TileContext(nc) as tc:
        with tc.tile_pool(name="const", bufs=1) as cpool, \
             tc.tile_pool(name="kprep", bufs=2) as kpool, \
             tc.tile_pool(name="knt", bufs=2) as ntpool, \
             tc.tile_pool(name="mp", bufs=2) as mpool, \
             tc.tile_pool(name="mask", bufs=3) as maskpool, \
             tc.tile_pool(name="sel", bufs=2) as selpool, \
             tc.tile_pool(name="io", bufs=1) as iopool, \
             tc.tile_pool(name="ps", bufs=8, space="PSUM") as psum:

            # ---------------- x prep: normalize, split, 2 transposes ----
            xr = cpool.tile([128, NBT * D], mybir.dt.float32)
            nc.sync.dma_start(out=xr[:, :], in_=x_d.ap())
            xsq = cpool.tile([128, NBT * D], mybir.dt.float32)
            nc.scalar.activation(xsq[:, :], xr[:, :], AF.Square)
            xss = cpool.tile([128, NBT], mybir.dt.float32)
            nc.vector.tensor_reduce(
                xss[:, :], xsq[:, :].rearrange("p (g d) -> p g d", g=NBT),
                axis=mybir.AxisListType.X, op=ALU.add,
                apply_absolute_value=False, negate=False)
            xsr = cpool.tile([128, NBT], mybir.dt.float32)
            nc.scalar.activation(xsr[:, :], xss[:, :], AF.Sqrt)
            xinv = cpool.tile([128, NBT], mybir.dt.float32)
            nc.vector.reciprocal(xinv[:, :], xsr[:, :])
            xn = cpool.tile([128, NBT * D], mybir.dt.float32)
            for g in range(NBT):
                nc.vector.tensor_scalar_mul(
                    xn[:, g * D:(g + 1) * D], xr[:, g * D:(g + 1) * D],
                    xinv[:, g:g + 1])
            xhl = cpool.tile([128, NBT * 128], mybir.dt.bfloat16)
            xlh = cpool.tile([128, NBT * 128], mybir.dt.bfloat16)
            xhl3 = xhl[:, :].rearrange("p (g e) -> p g e", g=NBT)
            xlh3 = xlh[:, :].rearrange("p (g e) -> p g e", g=NBT)
            xn3 = xn[:, :].rearrange("p (g d) -> p g d", g=NBT)
            nc.scalar.activation(xhl3[:, :, 0:D], xn3, AF.Copy)
            nc.gpsimd.tensor_sub(xhl3[:, :, D:128], xn3, xhl3[:, :, 0:D])
            nc.scalar.activation(xlh3[:, :, D:128], xn3, AF.Copy)
            nc.gpsimd.tensor_copy(xlh3[:, :, 0:D], xhl3[:, :, D:128])
            xa = cpool.tile([128, NBT * 128], mybir.dt.bfloat16)
            xb = cpool.tile([128, NBT * 128], mybir.dt.bfloat16)
            nc.sync.dma_start_transpose(
                xa[:, :].rearrange("p (t b) -> p t b", t=NBT), xhl[:, :])
            nc.sync.dma_start_transpose(
                xb[:, :].rearrange("p (t b) -> p t b", t=NBT), xlh[:, :])

            wsb = iopool.tile([128, QS * NBT * 64], mybir.dt.float32, tag="wout")
            csb = iopool.tile([128, QS * NBT], mybir.dt.float32, tag="cout")

            # ---------------- K prep (per q), emitted piecewise ---------
            def emit_kprep(q):
                """Returns list of thunks; call in order, spread over units."""
                kraw = kpool.tile([128, 32 * D], mybir.dt.float32, tag="kraw")
                ksq = kpool.tile([128, 32 * D], mybir.dt.float32, tag="ksq")
                kss = kpool.tile([128, 32], mybir.dt.float32, tag="kss")
                ksr = kpool.tile([128, 32], mybir.dt.float32, tag="ksr")
                kinv = kpool.tile([128, 32], mybir.dt.float32, tag="kinv")
                kn = kpool.tile([128, 32 * D], mybir.dt.float32, tag="kn")
                khl = kpool.tile([128, 32 * 128], mybir.dt.bfloat16, tag="khl")
                knt = ntpool.tile([128, MK], mybir.dt.bfloat16, tag="knt")
                mp = mpool.tile([128, 32 * U1], mybir.dt.float16, tag="mp")
                kn3 = kn[:, :].rearrange("p (g d) -> p g d", g=32)
                khl3 = khl[:, :].rearrange("p (g e) -> p g e", g=32)

                def t_dma():
                    nc.sync.dma_start(out=kraw[:, :], in_=k_d.ap()[q])
                    nc.sync.dma_start(out=mp[:, :], in_=mp_d.ap()[q])

                def t_sq():
                    nc.scalar.activation(ksq[:, :], kraw[:, :], AF.Square)

                def t_red():
                    nc.vector.tensor_reduce(
                        kss[:, :], ksq[:, :].rearrange("p (g d) -> p g d", g=32),
                        axis=mybir.AxisListType.X, op=ALU.add,
                        apply_absolute_value=False, negate=False)

                def t_inv():
                    nc.scalar.activation(ksr[:, :], kss[:, :], AF.Sqrt)
                    nc.vector.reciprocal(kinv[:, :], ksr[:, :])

                def t_scale(lo, hi):
                    def f():
                        for g in range(lo, hi):
                            nc.gpsimd.tensor_scalar_mul(
                                kn[:, g * D:(g + 1) * D],
                                kraw[:, g * D:(g + 1) * D], kinv[:, g:g + 1])
                    return f

                def t_hi():
                    nc.gpsimd.tensor_copy(khl3[:, :, 0:D], kn3)

                def t_lo():
                    nc.gpsimd.tensor_sub(khl3[:, :, D:128], kn3, khl3[:, :, 0:D])

                def t_tr():
                    nc.sync.dma_start_transpose(
                        knt[:, :].rearrange("p (t b) -> p t b", t=32), khl[:, :])

                thunks = [t_dma, t_sq, t_red, t_inv,
                          t_scale(0, 16), t_scale(16, 32), t_hi, t_lo, t_tr]
                return thunks, knt, mp

            # prologue: q0 prep fully
            th0, knt_q, mp_q = emit_kprep(0)
            for t in th0:
                t()

            def emit_mm2(pmT, pmp, wp):
                mT8 = pmT[:, :].bitcast(mybir.dt.float8e4)
                ks = []
                for t in range(16):
                    for j in range(2):
                        lhsT = mT8[:, 256 * t:256 * (t + 1)].rearrange(
                            "p (b two) -> p b two", two=2)[:, :, j:j + 1]
                        rhs = pmp[:, (t * 2 + j) * U1:(t * 2 + j + 1) * U1]
                        ks.append((lhsT, rhs))
                for k, (lhsT, rhs) in enumerate(ks):
                    nc.tensor.matmul(wp[:, :U1], lhsT, rhs,
                                     start=(k == 0), stop=(k == 31))

            def emit_epilogue(wp, uq, ubt):
                col = (uq * NBT + ubt)
                nc.scalar.activation(wsb[:, col * 64:(col + 1) * 64],
                                     wp[:, 0:64], AF.Copy, scale=1.0 / 16.0)
                nc.scalar.activation(csb[:, col:col + 1], wp[:, 64:65], AF.Copy)
                nc.sync.dma_start(out=w_d.ap()[:, col * 64:(col + 1) * 64],
                                  in_=wsb[:, col * 64:(col + 1) * 64])
                nc.sync.dma_start(out=cnt_d.ap()[:, col:col + 1],
                                  in_=csb[:, col:col + 1])

            pend = []
            next_thunks = None
            for u in range(QS * NBT):
                q, bt = u // NBT, u % NBT
                if bt == 0 and q + 1 < QS:
                    next_thunks, next_knt, next_mp = emit_kprep(q + 1)

                xau = xa[:, bt * 128:(bt + 1) * 128]
                xbu = xb[:, bt * 128:(bt + 1) * 128]
                chunks = []
                cands = selpool.tile([128, 64], mybir.dt.float32, tag="cands")
                for c in range(NCH):
                    rp = psum.tile([128, CH], mybir.dt.float32, tag="bank")
                    nc.tensor.matmul(rp[:, :], xau,
                                     knt_q[:, CH * c:CH * (c + 1)],
                                     start=True, stop=False)
                    nc.tensor.matmul(rp[:, :], xbu,
                                     knt_q[:, CH * c:CH * (c + 1)],
                                     start=False, stop=True)
                    nc.vector.max(cands[:, 8 * c:8 * (c + 1)], rp[:, :])
                    chunks.append(rp)
                    # spread next-q K prep across the unit's chunk slots
                    if next_thunks and bt * NCH + c < len(next_thunks) * 4 \
                       and (bt * NCH + c) % 4 == 3:
                        ti = (bt * NCH + c) // 4
                        if ti < len(next_thunks):
                            next_thunks[ti]()

                v1 = selpool.tile([128, 8], mybir.dt.float32, tag="v1")
                nc.vector.max(v1[:, :], cands[:, :])
                candr = selpool.tile([128, 64], mybir.dt.float32, tag="candr")
                nc.vector.match_replace(candr[:, :], v1[:, :], cands[:, :], -1e30)
                v2 = selpool.tile([128, 8], mybir.dt.float32, tag="v2")
                nc.vector.max(v2[:, :], candr[:, :])
                bts = selpool.tile([128, 1], mybir.dt.float32, tag="bts")
                nc.vector.tensor_scalar(bts[:, :], v2[:, 7:8], -SCALE, 37.0,
                                        op0=ALU.mult, op1=ALU.add)

                mask8 = maskpool.tile([128, MK], mybir.dt.float8e4, tag="mask8")
                for c in range(NCH):
                    nc.scalar.activation(
                        mask8[:, CH * c:CH * (c + 1)], chunks[c][:, :],
                        AF.Sigmoid, bias=bts[:, 0:1], scale=SCALE)
                    if c == NCH - 1 and len(pend) == 2:
                        # combine from two units ago into bank 7 (just masked);
                        # the next unit's mm1 claims bank 7 last.
                        pmT, pmp, puq, pubt = pend.pop(0)
                        emit_mm2(pmT, pmp, chunks[NCH - 1])
                        emit_epilogue(chunks[NCH - 1], puq, pubt)

                m16 = mask8[:, :].bitcast(mybir.dt.float16)
                mT = maskpool.tile([128, 2048], mybir.dt.float16, tag="maskT")
                nc.sync.dma_start_transpose(
                    mT[:, :].rearrange("p (t b) -> p t b", t=16), m16[:, :])
                pend.append((mT, mp_q, q, bt))

                if bt == NBT - 1 and next_thunks:
                    knt_q, mp_q = next_knt, next_mp
                    next_thunks = None

            for pmT, pmp, puq, pubt in pend:
                wp_last = psum.tile([128, CH], mybir.dt.float32, tag="bank")
                emit_mm2(pmT, pmp, wp_last)
                emit_epilogue(wp_last, puq, pubt)
    nc.compile()
    return nc


def _get(name, builder):
    if name not in _cache:
        _cache[name] = builder()
    return _cache[name]


# -------------------------------------------------------------- host fixup
def _fixup_rows(W, cnt, x, K, M):
    """Recompute rows whose on-device selection count != 16 with the exact
    reference formula (fp32)."""
    bad = np.argwhere(np.abs(cnt - 16.0) > 0.25)
    if len(bad) == 0:
        return W
    xf = np.asarray(x, np.float32)
    Kf = np.asarray(K, np.float32)
    Mf = np.asarray(M, np.float32)
    for b, q in bad:
        xb = xf[b]
        xb = xb / max(np.sqrt(np.sum(xb * xb)), 1e-12)
        Kq = Kf[q]
        nrm = np.maximum(np.sqrt(np.sum(Kq * Kq, axis=1)), 1e-12)
        r = (Kq @ xb) / nrm
        idx = np.argsort(-r, kind="stable")[:DELTA]
        tr = r[idx]
        a = np.exp(S_TEMP * (tr - tr.max()))
        a /= a.sum()
        W[b, q] = (a[:, None] * Mf[q][idx]).sum(0)
    return W


def _run_spmd(nc, in_maps, trace):
    try:
        return run_bass_kernel_spmd(nc, in_maps, core_ids=list(range(N_CORES)),
                                    trace=trace)
    except Exception:
        # transient NRT device errors recover on retry
        return run_bass_kernel_spmd(nc, in_maps, core_ids=list(range(N_CORES)),
                                    trace=trace)


# ------------------------------------------------------------------- main
def _run(x, K, M, trace=False):
    x = np.ascontiguousarray(np.asarray(x, np.float32))
    K = np.ascontiguousarray(np.asarray(K, np.float32))
    M = np.ascontiguousarray(np.asarray(M, np.float32))

    ncm = _get("m", _build)

    xr = x.reshape(128, NBT * D)                       # row 8p+g at (p, g)
    M16 = M.astype(np.float16)
    in_maps = []
    for c in range(N_CORES):
        Kc = K[c * QS:(c + 1) * QS].reshape(QS, 128, 32 * D)
        # Mp[q][p][c2*65+u] = M[qg][_MP_IDX[p, c2]][u], col 64 = 1.0
        Mg = M16[c * QS:(c + 1) * QS][:, _MP_IDX]      # [QS, 128, 32, 64]
        Mp = np.concatenate(
            [Mg, np.ones((QS, 128, 32, 1), np.float16)], axis=3
        ).reshape(QS, 128, 32 * U1)
        in_maps.append({"xr": xr, "Kc": np.ascontiguousarray(Kc),
                        "Mp": np.ascontiguousarray(Mp)})

    res = _run_spmd(ncm, in_maps, trace)

    W = np.empty((BF, Q, 64), np.float32)
    cnt = np.empty((BF, Q), np.float32)
    for c in range(N_CORES):
        wc = np.asarray(res.results[c]["W"], np.float32).reshape(128, QS, NBT, 64)
        cc = np.asarray(res.results[c]["cnt"], np.float32).reshape(128, QS, NBT)
        for bt in range(NBT):
            rows = 8 * np.arange(128) + bt             # batch = 8i + bt
            W[rows, c * QS:(c + 1) * QS] = wc[:, :, bt]
            cnt[rows, c * QS:(c + 1) * QS] = cc[:, :, bt]

    W = _fixup_rows(W, cnt, x, K, M)
    return W, res.exec_time_ns, 0


def kernel(x, K, M):
    W, _, _ = _run(x, K, M, trace=False)
    return W


# revision 11
# speedup vs baseline: 1.1614x; 1.0019x over previous
"""nn_CNUs kernel for 8 TRN2 NeuronCores — single merged q-sharded kernel.

Sharding: each core owns 4 of 32 q-neurons and processes ALL 1024 batch rows
(vs. the old batch-sharded 2-kernel pipeline that replicated 51MB of K/M DMA
per core and serialized normalize->host->combine).

Per core, per q: on-device L2-normalize K rows, split into interleaved
bf16 hi/lo [d_hi|d_lo] layout, xbar-transpose to [128, 4096] (contraction
layout). Per unit (q, 128-batch tile): responses via 2 stacked-bf16 matmuls
per 512-chunk (fp32-exact), DVE max8 screen -> top-16 threshold, masks via
ACT sigmoid / gpsimd is_ge into fp8, xbar mask transpose (SP queue), combine
mask @ [M|1] two units later interleaved into a just-masked PSUM bank.
Host does layout only (reshapes, fp16 cast, permutation gathers) + fixup of
rows whose selection count != 16 (ties/candidate misses, ~1e-4).
"""
import sys
if '/opt/trn_rl_repo' not in sys.path:
    sys.path.insert(0, '/opt/trn_rl_repo')

import numpy as np
import ml_dtypes

import concourse.bacc as bacc
import concourse.mybir as mybir
import concourse.tile as tile
from concourse.bass_utils import run_bass_kernel_spmd

N_CORES = 8
BF, D, Q, MK, DELTA = 1024, 64, 32, 4096, 16
QS = Q // N_CORES          # 4 q per core
NBT = 8                    # batch tiles of 128 per core
NCH, CH, U1 = 8, 512, 65
SCALE = float(2 ** 30)
S_TEMP = 0.1 / 8.0         # gamma_alpha / sqrt(D)
AF = mybir.ActivationFunctionType
ALU = mybir.AluOpType

_cache = {}

# knt column c holds K-row m_col(c) = 32*(c%128) + c//128 (from the
# contiguous [128p x 32 rows] SBUF fill + 128-blocked xbar transpose).
_MCOL = (32 * (np.arange(MK) % 128) + np.arange(MK) // 128).astype(np.int64)
# mm2 chunk c2 (=2t+j), partition p contracts mask fp8 column 2*(128*t+p)+j.
_t2 = np.arange(32) // 2
_j2 = np.arange(32) % 2
_MP_IDX = _MCOL[2 * (128 * _t2[None, :] + np.arange(128)[:, None]) + _j2[None, :]]


def _build():
    nc = bacc.Bacc("TRN2", target_bir_lowering=False, debug=False,
                   num_devices=N_CORES)
    x_d = nc.dram_tensor("xr", [128, NBT * D], mybir.dt.float32, kind="ExternalInput")
    k_d = nc.dram_tensor("Kc", [QS, 128, 32 * D], mybir.dt.float32, kind="ExternalInput")
    mp_d = nc.dram_tensor("Mp", [QS, 128, 32 * U1], mybir.dt.float16, kind="ExternalInput")
    w_d = nc.dram_tensor("W", [128, QS * NBT * 64], mybir.dt.float32, kind="ExternalOutput")
    cnt_d = nc.dram_tensor("cnt", [128, QS * NBT], mybir.dt.float32, kind="ExternalOutput")

    with tile.TileContext(nc) as tc:
        with tc.tile_pool(name="const", bufs=1) as cpool, \
             tc.tile_pool(name="kprep", bufs=2) as kpool, \
             tc.tile_pool(name="knt", bufs=2) as ntpool, \
             tc.tile_pool(name="mp", bufs=2) as mpool, \
             tc.tile_pool(name="mask", bufs=3) as maskpool, \
             tc.tile_pool(name="sel", bufs=2) as selpool, \
             tc.tile_pool(name="io", bufs=1) as iopool, \
             tc.tile_pool(name="ps", bufs=4, space="PSUM") as psum:

            # ---------------- x prep: normalize, split, 2 transposes ----
            xr = cpool.tile([128, NBT * D], mybir.dt.float32)
            nc.sync.dma_start(out=xr[:, :], in_=x_d.ap())
            xsq = cpool.tile([128, NBT * D], mybir.dt.float32)
            nc.scalar.activation(xsq[:, :], xr[:, :], AF.Square)
            xss = cpool.tile([128, NBT], mybir.dt.float32)
            nc.vector.tensor_reduce(
                xss[:, :], xsq[:, :].rearrange("p (g d) -> p g d", g=NBT),
                axis=mybir.AxisListType.X, op=ALU.add,
                apply_absolute_value=False, negate=False)
            xsr = cpool.tile([128, NBT], mybir.dt.float32)
            nc.scalar.activation(xsr[:, :], xss[:, :], AF.Sqrt)
            xinv = cpool.tile([128, NBT], mybir.dt.float32)
            nc.vector.reciprocal(xinv[:, :], xsr[:, :])
            xn = cpool.tile([128, NBT * D], mybir.dt.float32)
            for g in range(NBT):
                nc.vector.tensor_scalar_mul(
                    xn[:, g * D:(g + 1) * D], xr[:, g * D:(g + 1) * D],
                    xinv[:, g:g + 1])
            xhl = cpool.tile([128, NBT * 128], mybir.dt.bfloat16)
            xlh = cpool.tile([128, NBT * 128], mybir.dt.bfloat16)
            xhl3 = xhl[:, :].rearrange("p (g e) -> p g e", g=NBT)
            xlh3 = xlh[:, :].rearrange("p (g e) -> p g e", g=NBT)
            xn3 = xn[:, :].rearrange("p (g d) -> p g d", g=NBT)
            nc.scalar.activation(xhl3[:, :, 0:D], xn3, AF.Copy)
            nc.gpsimd.tensor_sub(xhl3[:, :, D:128], xn3, xhl3[:, :, 0:D])
            nc.scalar.activation(xlh3[:, :, D:128], xn3, AF.Copy)
            nc.gpsimd.tensor_copy(xlh3[:, :, 0:D], xhl3[:, :, D:128])
            xa = cpool.tile([128, NBT * 128], mybir.dt.bfloat16)
            xb = cpool.tile([128, NBT * 128], mybir.dt.bfloat16)
            nc.sync.dma_start_transpose(
                xa[:, :].rearrange("p (t b) -> p t b", t=NBT), xhl[:, :])
            nc.sync.dma_start_transpose(
                xb[:, :].rearrange("p (t b) -> p t b", t=NBT), xlh[:, :])

            wsb = iopool.tile([128, QS * NBT * 64], mybir.dt.float32, tag="wout")
            csb = iopool.tile([128, QS * NBT], mybir.dt.float32, tag="cout")

            # ---------------- K prep (per q), emitted piecewise ---------
            def emit_kprep(q):
                """Returns list of thunks; call in order, spread over units."""
                kraw = kpool.tile([128, 32 * D], mybir.dt.float32, tag="kraw")
                ksq = kpool.tile([128, 32 * D], mybir.dt.float32, tag="ksq")
                kss = kpool.tile([128, 32], mybir.dt.float32, tag="kss")
                ksr = kpool.tile([128, 32], mybir.dt.float32, tag="ksr")
                kinv = kpool.tile([128, 32], mybir.dt.float32, tag="kinv")
                kn = kpool.tile([128, 32 * D], mybir.dt.float32, tag="kn")
                khl = kpool.tile([128, 32 * 128], mybir.dt.bfloat16, tag="khl")
                knt = ntpool.tile([128, MK], mybir.dt.bfloat16, tag="knt")
                mp = mpool.tile([128, 32 * U1], mybir.dt.float16, tag="mp")
                kn3 = kn[:, :].rearrange("p (g d) -> p g d", g=32)
                khl3 = khl[:, :].rearrange("p (g e) -> p g e", g=32)

                def t_dma():
                    nc.sync.dma_start(out=kraw[:, :], in_=k_d.ap()[q])
                    nc.sync.dma_start(out=mp[:, :], in_=mp_d.ap()[q])

                def t_sq():
                    nc.gpsimd.tensor_mul(ksq[:, :], kraw[:, :], kraw[:, :])

                def t_red():
                    nc.vector.tensor_reduce(
                        kss[:, :], ksq[:, :].rearrange("p (g d) -> p g d", g=32),
                        axis=mybir.AxisListType.X, op=ALU.add,
                        apply_absolute_value=False, negate=False)

                def t_inv():
                    nc.scalar.activation(ksr[:, :], kss[:, :], AF.Sqrt)
                    nc.vector.reciprocal(kinv[:, :], ksr[:, :])

                def t_scale():
                    nc.vector.tensor_tensor(
                        kn3, kraw[:, :].rearrange("p (g d) -> p g d", g=32),
                        kinv[:, :].broadcast_to([128, 32, D]), op=ALU.mult)

                def t_hi():
                    nc.gpsimd.tensor_copy(khl3[:, :, 0:D], kn3)

                def t_lo():
                    nc.gpsimd.tensor_sub(khl3[:, :, D:128], kn3, khl3[:, :, 0:D])

                def t_tr():
                    nc.sync.dma_start_transpose(
                        knt[:, :].rearrange("p (t b) -> p t b", t=32), khl[:, :])

                thunks = [t_dma, t_sq, t_red, t_inv,
                          t_scale, t_hi, t_lo, t_tr]
                return thunks, knt, mp

            # prologue: q0 prep fully
            th0, knt_q, mp_q = emit_kprep(0)
            for t in th0:
                t()

            def emit_mm2(pmT, pmp, wp):
                mT8 = pmT[:, :].bitcast(mybir.dt.float8e4)
                ks = []
                for t in range(16):
                    for j in range(2):
                        lhsT = mT8[:, 256 * t:256 * (t + 1)].rearrange(
                            "p (b two) -> p b two", two=2)[:, :, j:j + 1]
                        rhs = pmp[:, (t * 2 + j) * U1:(t * 2 + j + 1) * U1]
                        ks.append((lhsT, rhs))
                for k, (lhsT, rhs) in enumerate(ks):
                    nc.tensor.matmul(wp[:, :U1], lhsT, rhs,
                                     start=(k == 0), stop=(k == 31))

            def emit_epilogue(wp, uq, ubt):
                col = (uq * NBT + ubt)
                nc.scalar.activation(wsb[:, col * 64:(col + 1) * 64],
                                     wp[:, 0:64], AF.Copy, scale=1.0 / 16.0)
                nc.scalar.activation(csb[:, col:col + 1], wp[:, 64:65], AF.Copy)
                nc.sync.dma_start(out=w_d.ap()[:, col * 64:(col + 1) * 64],
                                  in_=wsb[:, col * 64:(col + 1) * 64])
                nc.sync.dma_start(out=cnt_d.ap()[:, col:col + 1],
                                  in_=csb[:, col:col + 1])

            pend = []
            next_thunks = None
            for u in range(QS * NBT):
                q, bt = u // NBT, u % NBT
                if bt == 0 and q + 1 < QS:
                    next_thunks, next_knt, next_mp = emit_kprep(q + 1)

                xau = xa[:, bt * 128:(bt + 1) * 128]
                xbu = xb[:, bt * 128:(bt + 1) * 128]
                tiles = []
                cands = selpool.tile([128, 64], mybir.dt.float32, tag="cands")
                for c in range(NCH):
                    if c % 2 == 0:
                        rp = psum.tile([128, 2 * CH], mybir.dt.float32, tag="bank")
                        tiles.append(rp)
                    half = rp[:, (c % 2) * CH:(c % 2 + 1) * CH]
                    nc.tensor.matmul(half, xau,
                                     knt_q[:, CH * c:CH * (c + 1)],
                                     start=True, stop=False)
                    nc.tensor.matmul(half, xbu,
                                     knt_q[:, CH * c:CH * (c + 1)],
                                     start=False, stop=True)
                    nc.vector.max(cands[:, 8 * c:8 * (c + 1)], half)
                    # spread next-q K prep across the unit's chunk slots
                    if next_thunks and bt * NCH + c < len(next_thunks) * 4 \
                       and (bt * NCH + c) % 4 == 3:
                        ti = (bt * NCH + c) // 4
                        if ti < len(next_thunks):
                            next_thunks[ti]()

                v1 = selpool.tile([128, 8], mybir.dt.float32, tag="v1")
                nc.vector.max(v1[:, :], cands[:, :])
                candr = selpool.tile([128, 64], mybir.dt.float32, tag="candr")
                nc.vector.match_replace(candr[:, :], v1[:, :], cands[:, :], -1e30)
                v2 = selpool.tile([128, 8], mybir.dt.float32, tag="v2")
                nc.vector.max(v2[:, :], candr[:, :])
                bts = selpool.tile([128, 1], mybir.dt.float32, tag="bts")
                nc.vector.tensor_scalar(bts[:, :], v2[:, 7:8], -SCALE, 37.0,
                                        op0=ALU.mult, op1=ALU.add)

                mask8 = maskpool.tile([128, MK], mybir.dt.float8e4, tag="mask8")
                for c in range(4):
                    nc.scalar.activation(
                        mask8[:, 2 * CH * c:2 * CH * (c + 1)], tiles[c][:, :],
                        AF.Sigmoid, bias=bts[:, 0:1], scale=SCALE)
                    if c == 3 and len(pend) == 2:
                        # combine from two units ago into the last tile (just
                        # masked); the next unit's mm1 claims it last.
                        pmT, pmp, puq, pubt = pend.pop(0)
                        emit_mm2(pmT, pmp, tiles[3])
                        emit_epilogue(tiles[3], puq, pubt)

                m16 = mask8[:, :].bitcast(mybir.dt.float16)
                mT = maskpool.tile([128, 2048], mybir.dt.float16, tag="maskT")
                nc.sync.dma_start_transpose(
                    mT[:, :].rearrange("p (t b) -> p t b", t=16), m16[:, :])
                pend.append((mT, mp_q, q, bt))

                if bt == NBT - 1 and next_thunks:
                    knt_q, mp_q = next_knt, next_mp
                    next_thunks = None

            for pmT, pmp, puq, pubt in pend:
                wp_last = psum.tile([128, 2 * CH], mybir.dt.float32, tag="bank")
                emit_mm2(pmT, pmp, wp_last)
                emit_epilogue(wp_last, puq, pubt)
    nc.compile()
    return nc


def _get(name, builder):
    if name not in _cache:
        _cache[name] = builder()
    return _cache[name]


# -------------------------------------------------------------- host fixup
def _fixup_rows(W, cnt, x, K, M):
    """Recompute rows whose on-device selection count != 16 with the exact
    reference formula (fp32)."""
    bad = np.argwhere(np.abs(cnt - 16.0) > 0.25)
    if len(bad) == 0:
        return W
    xf = np.asarray(x, np.float32)
    Kf = np.asarray(K, np.float32)
    Mf = np.asarray(M, np.float32)
    for b, q in bad:
        xb = xf[b]
        xb = xb / max(np.sqrt(np.sum(xb * xb)), 1e-12)
        Kq = Kf[q]
        nrm = np.maximum(np.sqrt(np.sum(Kq * Kq, axis=1)), 1e-12)
        r = (Kq @ xb) / nrm
        idx = np.argsort(-r, kind="stable")[:DELTA]
        tr = r[idx]
        a = np.exp(S_TEMP * (tr - tr.max()))
        a /= a.sum()
        W[b, q] = (a[:, None] * Mf[q][idx]).sum(0)
    return W


def _run_spmd(nc, in_maps, trace):
    try:
        return run_bass_kernel_spmd(nc, in_maps, core_ids=list(range(N_CORES)),
                                    trace=trace)
    except Exception:
        # transient NRT device errors recover on retry
        return run_bass_kernel_spmd(nc, in_maps, core_ids=list(range(N_CORES)),
                                    trace=trace)


# ------------------------------------------------------------------- main
def _run(x, K, M, trace=False):
    x = np.ascontiguousarray(np.asarray(x, np.float32))
    K = np.ascontiguousarray(np.asarray(K, np.float32))
    M = np.ascontiguousarray(np.asarray(M, np.float32))

    ncm = _get("m", _build)

    xr = x.reshape(128, NBT * D)                       # row 8p+g at (p, g)
    M16 = M.astype(np.float16)
    in_maps = []
    for c in range(N_CORES):
        Kc = K[c * QS:(c + 1) * QS].reshape(QS, 128, 32 * D)
        # Mp[q][p][c2*65+u] = M[qg][_MP_IDX[p, c2]][u], col 64 = 1.0
        Mg = M16[c * QS:(c + 1) * QS][:, _MP_IDX]      # [QS, 128, 32, 64]
        Mp = np.concatenate(
            [Mg, np.ones((QS, 128, 32, 1), np.float16)], axis=3
        ).reshape(QS, 128, 32 * U1)
        in_maps.append({"xr": xr, "Kc": np.ascontiguousarray(Kc),
                        "Mp": np.ascontiguousarray(Mp)})

    res = _run_spmd(ncm, in_maps, trace)

    W = np.empty((BF, Q, 64), np.float32)
    cnt = np.empty((BF, Q), np.float32)
    for c in range(N_CORES):
        wc = np.asarray(res.results[c]["W"], np.float32).reshape(128, QS, NBT, 64)
        cc = np.asarray(res.results[c]["cnt"], np.float32).reshape(128, QS, NBT)
        for bt in range(NBT):
            rows = 8 * np.arange(128) + bt             # batch = 8i + bt
            W[rows, c * QS:(c + 1) * QS] = wc[:, :, bt]
            cnt[rows, c * QS:(c + 1) * QS] = cc[:, :, bt]

    W = _fixup_rows(W, cnt, x, K, M)
    return W, res.exec_time_ns, 0


def kernel(x, K, M):
    W, _, _ = _run(x, K, M, trace=False)
    return W


# revision 15
# speedup vs baseline: 1.2501x; 1.0764x over previous
"""nn_CNUs kernel for 8 TRN2 NeuronCores — single merged q-sharded kernel.

Sharding: each core owns 4 of 32 q-neurons and processes ALL 1024 batch rows
(vs. the old batch-sharded 2-kernel pipeline that replicated 51MB of K/M DMA
per core and serialized normalize->host->combine).

Per core, per q: on-device L2-normalize K rows, split into interleaved
bf16 hi/lo [d_hi|d_lo] layout, xbar-transpose to [128, 4096] (contraction
layout). Per unit (q, 128-batch tile): responses via 2 stacked-bf16 matmuls
per 512-chunk (fp32-exact), DVE max8 screen -> top-16 threshold, masks via
ACT sigmoid / gpsimd is_ge into fp8, xbar mask transpose (SP queue), combine
mask @ [M|1] two units later interleaved into a just-masked PSUM bank.
Host does layout only (reshapes, fp16 cast, permutation gathers) + fixup of
rows whose selection count != 16 (ties/candidate misses, ~1e-4).
"""
import sys
if '/opt/trn_rl_repo' not in sys.path:
    sys.path.insert(0, '/opt/trn_rl_repo')

import numpy as np
import ml_dtypes

import concourse.bacc as bacc
import concourse.mybir as mybir
import concourse.tile as tile
from concourse.bass_utils import run_bass_kernel_spmd

N_CORES = 8
BF, D, Q, MK, DELTA = 1024, 64, 32, 4096, 16
QS = Q // N_CORES          # 4 q per core
NBT = 8                    # batch tiles of 128 per core
NCH, CH, U1 = 8, 512, 65
SCALE = float(2 ** 30)
S_TEMP = 0.1 / 8.0         # gamma_alpha / sqrt(D)
AF = mybir.ActivationFunctionType
ALU = mybir.AluOpType

_cache = {}

# knt column c holds K-row m_col(c) = 32*(c%128) + c//128 (from the
# contiguous [128p x 32 rows] SBUF fill + 128-blocked xbar transpose).
_MCOL = (32 * (np.arange(MK) % 128) + np.arange(MK) // 128).astype(np.int64)
# mm2 chunk t, partition p contracts mask column 128*t+p (fp16 transpose).
_MP_IDX = _MCOL[128 * np.arange(32)[None, :] + np.arange(128)[:, None]]


def _build():
    nc = bacc.Bacc("TRN2", target_bir_lowering=False, debug=False,
                   num_devices=N_CORES)
    x_d = nc.dram_tensor("xr", [128, NBT * D], mybir.dt.float32, kind="ExternalInput")
    k_d = nc.dram_tensor("Kc", [QS, 128, 32 * D], mybir.dt.float32, kind="ExternalInput")
    mp_d = nc.dram_tensor("Mp", [QS, 128, 32 * U1], mybir.dt.float16, kind="ExternalInput")
    w_d = nc.dram_tensor("W", [128, QS * NBT * 64], mybir.dt.float32, kind="ExternalOutput")
    cnt_d = nc.dram_tensor("cnt", [128, QS * NBT], mybir.dt.float32, kind="ExternalOutput")

    with tile.TileContext(nc) as tc:
        with tc.tile_pool(name="const", bufs=1) as cpool, \
             tc.tile_pool(name="kprep", bufs=2) as kpool, \
             tc.tile_pool(name="knt", bufs=2) as ntpool, \
             tc.tile_pool(name="mp", bufs=2) as mpool, \
             tc.tile_pool(name="mask", bufs=3) as maskpool, \
             tc.tile_pool(name="sel", bufs=2) as selpool, \
             tc.tile_pool(name="io", bufs=1) as iopool, \
             tc.tile_pool(name="ps", bufs=4, space="PSUM") as psum:

            # ---------------- x prep: normalize, split, 2 transposes ----
            xr = cpool.tile([128, NBT * D], mybir.dt.float32)
            nc.sync.dma_start(out=xr[:, :], in_=x_d.ap())
            xsq = cpool.tile([128, NBT * D], mybir.dt.float32)
            nc.scalar.activation(xsq[:, :], xr[:, :], AF.Square)
            xss = cpool.tile([128, NBT], mybir.dt.float32)
            nc.vector.tensor_reduce(
                xss[:, :], xsq[:, :].rearrange("p (g d) -> p g d", g=NBT),
                axis=mybir.AxisListType.X, op=ALU.add,
                apply_absolute_value=False, negate=False)
            xsr = cpool.tile([128, NBT], mybir.dt.float32)
            nc.scalar.activation(xsr[:, :], xss[:, :], AF.Sqrt)
            xinv = cpool.tile([128, NBT], mybir.dt.float32)
            nc.vector.reciprocal(xinv[:, :], xsr[:, :])
            xn = cpool.tile([128, NBT * D], mybir.dt.float32)
            for g in range(NBT):
                nc.vector.tensor_scalar_mul(
                    xn[:, g * D:(g + 1) * D], xr[:, g * D:(g + 1) * D],
                    xinv[:, g:g + 1])
            xhl = cpool.tile([128, NBT * 128], mybir.dt.bfloat16)
            xlh = cpool.tile([128, NBT * 128], mybir.dt.bfloat16)
            xhl3 = xhl[:, :].rearrange("p (g e) -> p g e", g=NBT)
            xlh3 = xlh[:, :].rearrange("p (g e) -> p g e", g=NBT)
            xn3 = xn[:, :].rearrange("p (g d) -> p g d", g=NBT)
            nc.scalar.activation(xhl3[:, :, 0:D], xn3, AF.Copy)
            nc.gpsimd.tensor_sub(xhl3[:, :, D:128], xn3, xhl3[:, :, 0:D])
            nc.scalar.activation(xlh3[:, :, D:128], xn3, AF.Copy)
            nc.gpsimd.tensor_copy(xlh3[:, :, 0:D], xhl3[:, :, D:128])
            xa = cpool.tile([128, NBT * 128], mybir.dt.bfloat16)
            xb = cpool.tile([128, NBT * 128], mybir.dt.bfloat16)
            nc.sync.dma_start_transpose(
                xa[:, :].rearrange("p (t b) -> p t b", t=NBT), xhl[:, :])
            nc.sync.dma_start_transpose(
                xb[:, :].rearrange("p (t b) -> p t b", t=NBT), xlh[:, :])

            wsb = iopool.tile([128, QS * NBT * 64], mybir.dt.float32, tag="wout")
            csb = iopool.tile([128, QS * NBT], mybir.dt.float32, tag="cout")

            # ---------------- K prep (per q), emitted piecewise ---------
            def emit_kprep(q):
                """Returns list of thunks; call in order, spread over units."""
                kraw = kpool.tile([128, 32 * D], mybir.dt.float32, tag="kraw")
                ksq = kpool.tile([128, 32 * D], mybir.dt.float32, tag="ksq")
                kss = kpool.tile([128, 32], mybir.dt.float32, tag="kss")
                ksr = kpool.tile([128, 32], mybir.dt.float32, tag="ksr")
                kinv = kpool.tile([128, 32], mybir.dt.float32, tag="kinv")
                kn = kpool.tile([128, 32 * D], mybir.dt.float32, tag="kn")
                khl = kpool.tile([128, 32 * 128], mybir.dt.bfloat16, tag="khl")
                knt = ntpool.tile([128, MK], mybir.dt.bfloat16, tag="knt")
                mp = mpool.tile([128, 32 * U1], mybir.dt.float16, tag="mp")
                kn3 = kn[:, :].rearrange("p (g d) -> p g d", g=32)
                khl3 = khl[:, :].rearrange("p (g e) -> p g e", g=32)

                def t_dma():
                    nc.sync.dma_start(out=kraw[:, :], in_=k_d.ap()[q])
                    nc.sync.dma_start(out=mp[:, :], in_=mp_d.ap()[q])

                def t_sq():
                    nc.gpsimd.tensor_mul(ksq[:, :], kraw[:, :], kraw[:, :])

                def t_red():
                    nc.vector.tensor_reduce(
                        kss[:, :], ksq[:, :].rearrange("p (g d) -> p g d", g=32),
                        axis=mybir.AxisListType.X, op=ALU.add,
                        apply_absolute_value=False, negate=False)

                def t_inv():
                    nc.scalar.activation(ksr[:, :], kss[:, :], AF.Sqrt)
                    nc.vector.reciprocal(kinv[:, :], ksr[:, :])

                def t_scale():
                    nc.vector.tensor_tensor(
                        kn3, kraw[:, :].rearrange("p (g d) -> p g d", g=32),
                        kinv[:, :].broadcast_to([128, 32, D]), op=ALU.mult)

                def t_hi():
                    nc.gpsimd.tensor_copy(khl3[:, :, 0:D], kn3)

                def t_lo():
                    nc.gpsimd.tensor_sub(khl3[:, :, D:128], kn3, khl3[:, :, 0:D])

                def t_tr():
                    nc.sync.dma_start_transpose(
                        knt[:, :].rearrange("p (t b) -> p t b", t=32), khl[:, :])

                thunks = [t_dma, t_sq, t_red, t_inv,
                          t_scale, t_hi, t_lo, t_tr]
                return thunks, knt, mp

            # prologue: q0 prep fully
            th0, knt_q, mp_q = emit_kprep(0)
            for t in th0:
                t()

            def emit_mm2(pmT, pmp, wp):
                for t in range(32):
                    nc.tensor.matmul(wp[:, :U1],
                                     pmT[:, 128 * t:128 * (t + 1)],
                                     pmp[:, t * U1:(t + 1) * U1],
                                     start=(t == 0), stop=(t == 31))

            def emit_epilogue(wp, uq, ubt):
                col = (uq * NBT + ubt)
                nc.scalar.activation(wsb[:, col * 64:(col + 1) * 64],
                                     wp[:, 0:64], AF.Copy, scale=1.0 / 16.0)
                nc.scalar.activation(csb[:, col:col + 1], wp[:, 64:65], AF.Copy)
                nc.sync.dma_start(out=w_d.ap()[:, col * 64:(col + 1) * 64],
                                  in_=wsb[:, col * 64:(col + 1) * 64])
                nc.sync.dma_start(out=cnt_d.ap()[:, col:col + 1],
                                  in_=csb[:, col:col + 1])

            pend = []
            next_thunks = None
            for u in range(QS * NBT):
                q, bt = u // NBT, u % NBT
                if bt == 0 and q + 1 < QS:
                    next_thunks, next_knt, next_mp = emit_kprep(q + 1)

                xau = xa[:, bt * 128:(bt + 1) * 128]
                xbu = xb[:, bt * 128:(bt + 1) * 128]
                tiles = []
                # fp16 copy of the responses: releases PSUM (PE free-runs);
                # screen/threshold/mask all read it. fp16 rounding is
                # monotone, so a count==16 selection is exactly the true
                # top-16; boundary ties give count!=16 -> host fixup.
                rcp = maskpool.tile([128, MK], mybir.dt.float16, tag="rcp")
                cands = selpool.tile([128, 32], mybir.dt.float16, tag="cands")
                for c in range(NCH):
                    if c % 2 == 0:
                        rp = psum.tile([128, 2 * CH], mybir.dt.float32, tag="bank")
                        tiles.append(rp)
                    half = rp[:, (c % 2) * CH:(c % 2 + 1) * CH]
                    nc.tensor.matmul(half, xau,
                                     knt_q[:, CH * c:CH * (c + 1)],
                                     start=True, stop=False)
                    nc.tensor.matmul(half, xbu,
                                     knt_q[:, CH * c:CH * (c + 1)],
                                     start=False, stop=True)
                    if c % 2 == 1:
                        tn = c // 2
                        nc.scalar.activation(
                            rcp[:, 2 * CH * tn:2 * CH * (tn + 1)],
                            rp[:, :], AF.Copy)
                        nc.vector.max(cands[:, 8 * tn:8 * (tn + 1)],
                                      rcp[:, 2 * CH * tn:2 * CH * (tn + 1)])
                    # spread next-q K prep across the unit's chunk slots
                    if next_thunks and bt * NCH + c < len(next_thunks) * 4 \
                       and (bt * NCH + c) % 4 == 3:
                        ti = (bt * NCH + c) // 4
                        if ti < len(next_thunks):
                            next_thunks[ti]()

                # combine from two units ago; tile 3 was released by its copy.
                if len(pend) == 2:
                    pmT, pmp, puq, pubt = pend.pop(0)
                    emit_mm2(pmT, pmp, tiles[3])
                    emit_epilogue(tiles[3], puq, pubt)

                v1 = selpool.tile([128, 8], mybir.dt.float16, tag="v1")
                nc.vector.max(v1[:, :], cands[:, :])
                candr = selpool.tile([128, 32], mybir.dt.float16, tag="candr")
                nc.vector.match_replace(candr[:, :], v1[:, :], cands[:, :],
                                        -60000.0)
                v2 = selpool.tile([128, 8], mybir.dt.float16, tag="v2")
                nc.vector.max(v2[:, :], candr[:, :])
                t32 = selpool.tile([128, 1], mybir.dt.float32, tag="t32")
                nc.vector.tensor_copy(t32[:, :], v2[:, 7:8])

                mask16 = maskpool.tile([128, MK], mybir.dt.float16, tag="mask16")
                for c in range(4):
                    nc.vector.tensor_scalar(
                        mask16[:, 2 * CH * c:2 * CH * (c + 1)],
                        rcp[:, 2 * CH * c:2 * CH * (c + 1)],
                        t32[:, 0:1], None, op0=ALU.is_ge)

                mT = maskpool.tile([128, MK], mybir.dt.float16, tag="maskT")
                nc.sync.dma_start_transpose(
                    mT[:, :].rearrange("p (t b) -> p t b", t=32), mask16[:, :])
                pend.append((mT, mp_q, q, bt))

                if bt == NBT - 1 and next_thunks:
                    knt_q, mp_q = next_knt, next_mp
                    next_thunks = None

            for pmT, pmp, puq, pubt in pend:
                wp_last = psum.tile([128, 2 * CH], mybir.dt.float32, tag="bank")
                emit_mm2(pmT, pmp, wp_last)
                emit_epilogue(wp_last, puq, pubt)
    nc.compile()
    return nc


def _get(name, builder):
    if name not in _cache:
        _cache[name] = builder()
    return _cache[name]


# -------------------------------------------------------------- host fixup
def _fixup_rows(W, cnt, x, K, M):
    """Recompute rows whose on-device selection count != 16 with the exact
    reference formula (fp32), batched per q."""
    bad = np.argwhere(np.abs(cnt - 16.0) > 0.25)
    if len(bad) == 0:
        return W
    xf = np.asarray(x, np.float32)
    xn = xf / np.maximum(np.sqrt((xf * xf).sum(1, keepdims=True)), 1e-12)
    Kf = np.asarray(K, np.float32)
    Mf = np.asarray(M, np.float32)
    for q in np.unique(bad[:, 1]):
        bs = bad[bad[:, 1] == q, 0]
        Kq = Kf[q]
        nrm = np.maximum(np.sqrt((Kq * Kq).sum(1)), 1e-12)
        r = (xn[bs] @ Kq.T) / nrm                       # [nb, MK]
        part = np.argpartition(-r, DELTA - 1, axis=1)[:, :DELTA]
        tr = np.take_along_axis(r, part, 1)
        ordr = np.argsort(-tr, axis=1, kind="stable")
        idx = np.take_along_axis(part, ordr, 1)         # sorted top-16
        tr = np.take_along_axis(tr, ordr, 1)
        a = np.exp(S_TEMP * (tr - tr.max(1, keepdims=True)))
        a /= a.sum(1, keepdims=True)
        W[bs, q] = np.einsum("nk,nku->nu", a, Mf[q][idx])
    return W


def _run_spmd(nc, in_maps, trace):
    try:
        return run_bass_kernel_spmd(nc, in_maps, core_ids=list(range(N_CORES)),
                                    trace=trace)
    except Exception:
        # transient NRT device errors recover on retry
        return run_bass_kernel_spmd(nc, in_maps, core_ids=list(range(N_CORES)),
                                    trace=trace)


# ------------------------------------------------------------------- main
def _run(x, K, M, trace=False):
    x = np.ascontiguousarray(np.asarray(x, np.float32))
    K = np.ascontiguousarray(np.asarray(K, np.float32))
    M = np.ascontiguousarray(np.asarray(M, np.float32))

    ncm = _get("m", _build)

    xr = x.reshape(128, NBT * D)                       # row 8p+g at (p, g)
    M16 = M.astype(np.float16)
    in_maps = []
    for c in range(N_CORES):
        Kc = K[c * QS:(c + 1) * QS].reshape(QS, 128, 32 * D)
        # Mp[q][p][c2*65+u] = M[qg][_MP_IDX[p, c2]][u], col 64 = 1.0
        Mg = M16[c * QS:(c + 1) * QS][:, _MP_IDX]      # [QS, 128, 32, 64]
        Mp = np.concatenate(
            [Mg, np.ones((QS, 128, 32, 1), np.float16)], axis=3
        ).reshape(QS, 128, 32 * U1)
        in_maps.append({"xr": xr, "Kc": np.ascontiguousarray(Kc),
                        "Mp": np.ascontiguousarray(Mp)})

    res = _run_spmd(ncm, in_maps, trace)

    W = np.empty((BF, Q, 64), np.float32)
    cnt = np.empty((BF, Q), np.float32)
    for c in range(N_CORES):
        wc = np.asarray(res.results[c]["W"], np.float32).reshape(128, QS, NBT, 64)
        cc = np.asarray(res.results[c]["cnt"], np.float32).reshape(128, QS, NBT)
        for bt in range(NBT):
            rows = 8 * np.arange(128) + bt             # batch = 8i + bt
            W[rows, c * QS:(c + 1) * QS] = wc[:, :, bt]
            cnt[rows, c * QS:(c + 1) * QS] = cc[:, :, bt]

    W = _fixup_rows(W, cnt, x, K, M)
    return W, res.exec_time_ns, 0


def kernel(x, K, M):
    W, _, _ = _run(x, K, M, trace=False)
    return W
